# revision 33
# baseline (speedup 1.0000x reference)
"""EnhanceSelfAttention (B=2, S=2048, C=1024, H=16, D=64) on 8 trn2 cores.

Sharding: core c -> batch b = c // 4, head group g = c % 4 (heads 4g..4g+3).
Each core computes its 4 heads end-to-end plus a partial output projection
(rows of w_out for its heads); host sums the 4 fp16 partials per batch.

Per-core dataflow (fp16 matmul operands, fp32 PSUM accumulation), organized
as one flat software pipeline that keeps the PE and ACT engines co-busy:

  - Startup: inputs are pre-arranged partition-major on the host so every
    load is a cheap 2D DMA, and are spread across the three DMA rings
    (sync/gpsimd/scalar, ~80-120GB/s each) in compute-deadline order; the
    first QKV matmul issues ~18us in, the first softmax exp ~45us in.
  - QKV projection from partition-major x: qT/kT produced transposed
    [cols, S] (pair-split layout via host-permuted weight columns so RoPE
    needs only a 32-row block swap), V natural [S, 4*65] with a ones
    column per head (folded into the projection) that accumulates the
    softmax denominator.  RoPE on DVE: rot = q*cos + swap32(q)*sinS.
  - Attention runs as a single flat pipeline over both head pairs and all
    q-chunks: scores (two 64-row tile_position matmuls run concurrently
    on the PE), causal mask on the diagonal block via identity-matmul,
    exp on ACT (scale=1/8 fused) with a 2-step lookahead ahead of the PV
    matmuls so chunk boundaries never starve ACT; softmax normalization
    is a fast reciprocal + fp16 gpsimd partition broadcast whose attnT
    multiplies are deferred one step (no DVE head-of-line blocking).
  - All remaining projection work executes as deadline-ordered fill units
    inside the attention stream (pair-0 seq-half-1 Q/K, V tiles 8-15,
    pair-1 Q/K, then per-chunk out-projection); y is written fp16 per
    128-row tile on the gpsimd ring and the host sums 4 partials/batch.
"""
import sys

if "/opt/trn_rl_repo" not in sys.path:
    sys.path.insert(0, "/opt/trn_rl_repo")

from collections import deque
from functools import partial

import numpy as np

import concourse.bacc as bacc
import concourse.bass as bass
import concourse.tile as tile
from concourse import mybir
from concourse.bass_utils import run_bass_kernel_spmd

B, S, C = 2, 2048, 1024
H, D = 16, 64
TEMP = 1e4
N_CORES = 8
HPC = 4            # heads per core
P = 128
NQC = S // 512     # 4 q-chunks of 512
KT = S // P        # 16 k-tiles
CKT = C // P       # 8 contraction tiles for projections

f32 = mybir.dt.float32
bf16 = mybir.dt.bfloat16
fp16 = mybir.dt.float16

_NC = None


def _build():
    nc = bacc.Bacc("TRN2", target_bir_lowering=False, debug=False)

    xT = nc.dram_tensor("xT", [P, 2, CKT, 1024], fp16, kind="ExternalInput").ap()
    wq = nc.dram_tensor("wq", [P, CKT, 256], fp16, kind="ExternalInput").ap()
    wk = nc.dram_tensor("wk", [P, CKT, 256], fp16, kind="ExternalInput").ap()
    wv = nc.dram_tensor("wv", [P, CKT, 260], fp16, kind="ExternalInput").ap()
    wo = nc.dram_tensor("wo", [P, 2, C], fp16, kind="ExternalInput").ap()
    qb = nc.dram_tensor("qb", [2, P, 1], f32, kind="ExternalInput").ap()
    vb = nc.dram_tensor("vb", [1, 260], f32, kind="ExternalInput").ap()
    cosT = nc.dram_tensor("cosT", [P, S], fp16, kind="ExternalInput").ap()
    sinT = nc.dram_tensor("sinT", [P, S], fp16, kind="ExternalInput").ap()
    tri = nc.dram_tensor("tri", [P, P], bf16, kind="ExternalInput").ap()
    idn = nc.dram_tensor("idn", [P, P], bf16, kind="ExternalInput").ap()
    y = nc.dram_tensor("y", [S, C], fp16, kind="ExternalOutput").ap()

    with tile.TileContext(nc) as tc:
        _body(nc, tc, xT, wq, wk, wv, wo, qb, vb, cosT, sinT, tri, idn, y)
    nc.compile()
    return nc


def _body(nc, tc, xT, wq, wk, wv, wo, qb, vb, cosT, sinT, tri, idn, y):
    from contextlib import ExitStack

    with ExitStack() as ctx:
        consts = ctx.enter_context(tc.tile_pool(name="consts", bufs=1))
        # PSUM budget (8 banks): scores 2x[128,1024] = 4, shared
        # projection/PV rotation 3x[128,512] = 3, fill slot 1x[128,512] = 1.
        scp = ctx.enter_context(tc.tile_pool(name="scp", bufs=2, space="PSUM"))
        pvp = ctx.enter_context(tc.tile_pool(name="pvp", bufs=3, space="PSUM"))
        projp = ctx.enter_context(tc.tile_pool(name="projp", bufs=1, space="PSUM"))
        basep = ctx.enter_context(tc.tile_pool(name="basep", bufs=2))
        swp = ctx.enter_context(tc.tile_pool(name="swp", bufs=2))
        exp_pool = ctx.enter_context(tc.tile_pool(name="expool", bufs=6))
        rcp = ctx.enter_context(tc.tile_pool(name="rcp", bufs=2))
        rbp = ctx.enter_context(tc.tile_pool(name="rbp", bufs=2))
        yop = ctx.enter_context(tc.tile_pool(name="yop", bufs=3))

        wq_sb = consts.tile([P, CKT, 256], fp16, tag="wq", name="wq")
        wk_sb = consts.tile([P, CKT, 256], fp16, tag="wk", name="wk")
        wv_sb = consts.tile([P, CKT, 260], fp16, tag="wv", name="wv")
        wo_sb = consts.tile([P, 2, C], fp16, tag="wo", name="wo")
        cos_sb = consts.tile([P, S], fp16, tag="cos", name="cos")
        sin_sb = consts.tile([P, S], fp16, tag="sin", name="sin")
        tri_sb = consts.tile([P, P], bf16, tag="tri", name="tri")
        idn_sb = consts.tile([P, P], bf16, tag="idn", name="idn")
        qb_sb = [consts.tile([P, 1], f32, tag=f"qb{t}", name=f"qb{t}") for t in range(2)]
        vb_sb = consts.tile([P, 260], f32, tag="vb", name="vb")
        xq = [[consts.tile([P, 4, 1024], fp16, tag=f"x{h}{g}", name=f"x{h}{g}")
               for g in (0, 1)] for h in (0, 1)]

        def x_mv(k, s0, width=512):
            # moving slice of x for contraction tile k, seq start s0
            return xq[s0 // 1024][k // 4][:, k % 4, s0 % 1024:s0 % 1024 + width]

        qrot = [consts.tile([P, S], fp16, tag=f"qrot{t}", name=f"qrot{t}") for t in range(2)]
        krot = [consts.tile([P, S], fp16, tag=f"krot{t}", name=f"krot{t}") for t in range(2)]
        v_sb = [consts.tile([P, 260], fp16, tag=f"v{st}", name=f"v{st}") for st in range(KT)]
        attnT = [consts.tile([P, S], fp16, tag=f"attnT{t}", name=f"attnT{t}") for t in range(2)]

        # ---------------- DMA issue (3 rings, compute order) -------------
        # x split across the sync and gpsimd rings (half 0 of each first so
        # QKV can start early); weights + tables on the scalar ring, whose
        # issue work finishes ~25us before the first exp needs the queue.
        # sync ring: x k0-3 half 0, then wk, cos half 0, x half 1, cos half 1
        nc.sync.dma_start(xq[0][0][:], xT[:, 0, 0:4, :])
        nc.sync.dma_start(wk_sb[:], wk[:])
        nc.sync.dma_start(cos_sb[:, 0:1024], cosT[:, 0:1024])
        nc.sync.dma_start(xq[1][0][:], xT[:, 1, 0:4, :])
        nc.sync.dma_start(cos_sb[:, 1024:2048], cosT[:, 1024:2048])
        # gpsimd ring: x k4-7 half 0, then wv, sin half 0, x half 1, sin half 1
        nc.gpsimd.dma_start(xq[0][1][:], xT[:, 0, 4:8, :])
        nc.gpsimd.dma_start(wv_sb[:], wv[:])
        nc.gpsimd.dma_start(sin_sb[:, 0:1024], sinT[:, 0:1024])
        nc.gpsimd.dma_start(xq[1][1][:], xT[:, 1, 4:8, :])
        nc.gpsimd.dma_start(sin_sb[:, 1024:2048], sinT[:, 1024:2048])
        # scalar ring (slowest): wq first, then the small consts and wo
        nc.scalar.dma_start(wq_sb[:], wq[:])
        for t in range(2):
            nc.scalar.dma_start(qb_sb[t][:], qb[t])
        nc.scalar.dma_start(
            vb_sb[:],
            bass.AP(tensor=vb.tensor, offset=vb.offset, ap=[[0, P], [1, 260]]),
        )
        nc.scalar.dma_start(tri_sb[:], tri[:])
        nc.scalar.dma_start(idn_sb[:], idn[:])
        nc.scalar.dma_start(wo_sb[:], wo[:])

        # ---------------- QKV projection + RoPE helpers ------------------
        def qk_mm(kind, t, half, n2, klo, pool, state):
            """4 accumulation matmuls; allocates psum at klo==0, finishes
            the group + base copy at klo==4 end (n2 half of a 1024 span)."""
            wsb = wq_sb if kind == "q" else wk_sb
            if n2 == 0 and klo == 0:
                state["base"] = basep.tile([P, 1024], fp16, tag="base", name="base")
            if klo == 0:
                state["ps"] = pool.tile([P, 512], f32, tag=pool_tag[id(pool)], name="qkps")
            ps = state["ps"]
            base = state["base"]
            s0 = half * 1024 + n2 * 512
            for k in range(klo, klo + 4):
                nc.tensor.matmul(
                    ps[:],
                    wsb[:, k, t * P:(t + 1) * P],
                    x_mv(k, s0),
                    start=(k == 0), stop=(k == CKT - 1),
                )
            if klo == 4:
                if kind == "q":
                    nc.vector.tensor_scalar_add(
                        base[:, n2 * 512:(n2 + 1) * 512], ps[:], qb_sb[t][:, 0:1]
                    )
                else:
                    nc.vector.tensor_copy(base[:, n2 * 512:(n2 + 1) * 512], ps[:])
                if n2 == 1:
                    sw = swp.tile([P, 1024], fp16, tag="sw", name="sw")
                    state["sw"] = sw
                    for blk in range(4):
                        sb_, db_ = 32 * (blk ^ 1), 32 * blk
                        nc.sync.dma_start(sw[db_:db_ + 32, :], base[sb_:sb_ + 32, :])

        def qk_rope(kind, t, half, state):
            rot = qrot if kind == "q" else krot
            span = slice(half * 1024, (half + 1) * 1024)
            base, sw = state["base"], state["sw"]
            nc.vector.tensor_mul(base[:], base[:], cos_sb[:, span])
            nc.vector.tensor_mul(sw[:], sw[:], sin_sb[:, span])
            nc.vector.tensor_add(rot[t][:, span], base[:], sw[:])

        def qk_group(kind, t, half, pool):
            state = {}
            for n2 in (0, 1):
                for klo in (0, 4):
                    qk_mm(kind, t, half, n2, klo, pool, state)
            qk_rope(kind, t, half, state)

        def v_group(st, pool):
            ps = pool.tile([P, 260], f32, tag=pool_tag[id(pool)], name="vps")
            for k in range(CKT):
                nc.tensor.matmul(
                    ps[:],
                    x_mv(k, st * P, P),
                    wv_sb[:, k, :],
                    start=(k == 0), stop=(k == CKT - 1),
                )
            nc.vector.tensor_add(v_sb[st][:], ps[:], vb_sb[:])

        # ---------------- attention pieces -------------------------------
        def scores_exp(t, n, k):
            diag = (k // 4 == n)
            off = P * (k % 4) if diag else 0
            sc = scp.tile([P, 1024], f32, tag="sc", name="sc")
            for hh in (0, 1):
                nc.tensor.matmul(
                    sc[:, hh * 512 + off:(hh + 1) * 512],
                    krot[t][64 * hh:64 * hh + 64, k * P:(k + 1) * P],
                    qrot[t][64 * hh:64 * hh + 64, n * 512 + off:(n + 1) * 512],
                    start=True, stop=not diag,
                    tile_position=(64 * hh, 0),
                )
            if diag:
                for hh in (0, 1):
                    nc.tensor.matmul(
                        sc[:, hh * 512 + off:hh * 512 + off + P],
                        idn_sb[:], tri_sb[:],
                        start=False, stop=True,
                    )
            ex = exp_pool.tile([P, 1024], fp16, tag="ex", name="ex")
            if diag:
                exr = ex[:].rearrange("p (h c) -> p h c", h=2)[:, :, off:]
                scr = sc[:].rearrange("p (h c) -> p h c", h=2)[:, :, off:]
                nc.scalar.activation(
                    exr, scr, mybir.ActivationFunctionType.Exp, scale=0.125
                )
            else:
                nc.scalar.activation(
                    ex[:], sc[:], mybir.ActivationFunctionType.Exp, scale=0.125
                )
            return ex, off

        def pv_step(t, pv, klast, k, ex, off):
            for hh in (0, 1):
                h = 2 * t + hh
                nc.tensor.matmul(
                    pv[hh][:, off:512],
                    v_sb[k][:, 65 * h:65 * h + 65],
                    ex[:, hh * 512 + off:(hh + 1) * 512],
                    start=(k == 0), stop=(k == klast),
                )

        def normalize_part1(pv):
            rbs = []
            for hh in (0, 1):
                dc = rcp.tile([1, 512], f32, tag="dc", name="dc")
                nc.vector.tensor_copy(dc[:], pv[hh][64:65, :])
                rc = rcp.tile([1, 512], f32, tag="rc", name="rc")
                nc.vector.reciprocal_approx_fast(rc[:], dc[:])
                rch = rcp.tile([1, 512], fp16, tag="rch", name="rch")
                nc.vector.tensor_copy(rch[:], rc[:])
                rb = rbp.tile([64, 512], fp16, tag="rb", name="rb")
                nc.gpsimd.partition_broadcast(rb[:], rch[0:1, :])
                rbs.append(rb)
            return rbs

        def normalize_part2(t, n, pv, rbs):
            span = slice(n * 512, (n + 1) * 512)
            for hh in (0, 1):
                nc.vector.tensor_mul(
                    attnT[t][64 * hh:64 * hh + 64, span], pv[hh][0:64, :],
                    rbs[hh][:]
                )

        # ---------------- output projection ------------------------------
        def outproj_unit(st, c2, state, pool):
            if c2 == 0:
                state["yo"] = yop.tile([P, C], fp16, tag="yo", name="yo")
            yo = state["yo"]
            po = pool.tile([P, 512], f32, tag=pool_tag[id(pool)], name="po")
            for kk in (0, 1):
                nc.tensor.matmul(
                    po[:],
                    attnT[kk][:, st * P:(st + 1) * P],
                    wo_sb[:, kk, c2 * 512:(c2 + 1) * 512],
                    start=(kk == 0), stop=(kk == 1),
                )
            if pool is pvp:
                # drain phase: ACT is idle after the last exp
                nc.scalar.activation(yo[:, c2 * 512:(c2 + 1) * 512], po[:],
                                     mybir.ActivationFunctionType.Copy)
            else:
                nc.vector.tensor_copy(yo[:, c2 * 512:(c2 + 1) * 512], po[:])
            if c2 == 1:
                nc.gpsimd.dma_start(y[st * P:(st + 1) * P, :], yo[:])

        pool_tag = {id(pvp): "pv", id(projp): "proj"}

        # ---------------- schedule ---------------------------------------
        # Phase A (halved): seq-half 0 of pair-0 Q/K plus V tiles 0-7, so
        # the softmax pipeline starts ~20us earlier; the rest is
        # deadline-ordered fill work inside the attention stream.
        # both groups' matmuls + swap DMAs first, ropes deferred so each
        # swap round-trip hides behind the other group's DVE work
        stq, stk = {}, {}
        for n2 in (0, 1):
            for klo in (0, 4):
                qk_mm("q", 0, 0, n2, klo, pvp, stq)
        warm = pvp.tile([P, 512], f32, tag="pv", name="warm")
        for w_ in range(12):
            nc.tensor.matmul(
                warm[:], wq_sb[:, w_ % CKT, 0:P], x_mv(w_ % CKT, 0),
                start=True, stop=True,
            )
        for n2 in (0, 1):
            for klo in (0, 4):
                qk_mm("k", 0, 0, n2, klo, pvp, stk)
        qk_rope("q", 0, 0, stq)
        qk_rope("k", 0, 0, stk)
        for st in range(8):
            v_group(st, pvp)

        def qk_units(kind, t, half):
            state = {}
            units = []
            for n2 in (0, 1):
                for klo in (0, 4):
                    units.append(
                        partial(qk_mm, kind, t, half, n2, klo, projp, state))
            units.append(partial(qk_rope, kind, t, half, state))
            return units

        fill_queue = deque()
        # deadline order (fill slot i ~ attention step i+4): pair-0 half 1
        # before scores step 12, v8-11 before their PV retires (~step 22),
        # pair-1 half 0 before scores step 40, v12-15 before ~step 34.
        fill_queue.extend(qk_units("q", 0, 1) + qk_units("k", 0, 1))
        for st in range(8, 12):
            fill_queue.append(partial(v_group, st, projp))
        fill_queue.extend(qk_units("q", 1, 0) + qk_units("k", 1, 0))
        for st in range(12, KT):
            fill_queue.append(partial(v_group, st, projp))
        fill_queue.extend(qk_units("q", 1, 1) + qk_units("k", 1, 1))

        def fill_fn():
            if fill_queue:
                fill_queue.popleft()()

        # Flat attention pipeline across both pairs and all chunks with a
        # 2-step scores/exp lookahead so chunk boundaries never starve ACT.
        steps = [(t, n, k) for t in (0, 1) for n in range(NQC)
                 for k in range(4 * n + 4)]
        pv_tiles = {}
        window = deque()
        pending_finish = deque()

        def finish_one():
            (pt, pn, pv, rbs) = pending_finish.popleft()
            normalize_part2(pt, pn, pv, rbs)
            if pt == 1:
                for st in range(4 * pn, 4 * pn + 4):
                    state = {}
                    for c2 in (0, 1):
                        fill_queue.append(
                            partial(outproj_unit, st, c2, state, projp))

        def retire_one():
            if pending_finish:
                finish_one()
            (pt, pn, pk, pex, poff) = window.popleft()
            pv = pv_tiles.pop((pt, pn)) if pk == 4 * pn + 3 else pv_tiles[(pt, pn)]
            pv_step(pt, pv, 4 * pn + 3, pk, pex, poff)
            if pk == 4 * pn + 3:
                rbs = normalize_part1(pv)
                pending_finish.append((pt, pn, pv, rbs))

        for i, (t, n, k) in enumerate(steps):
            if (t, n) not in pv_tiles:
                pv_tiles[(t, n)] = [
                    pvp.tile([65, 512], f32, tag="pv", name=f"pv{hh}")
                    for hh in (0, 1)
                ]
            ex, off = scores_exp(t, n, k)
            if i >= 4:
                fill_fn()
                if len(fill_queue) > len(steps) - i:
                    fill_fn()
            if len(window) >= 2:
                retire_one()
            window.append((t, n, k, ex, off))
        while window:
            retire_one()
        while pending_finish:
            finish_one()
        # tail out-projection drains through the 3-slot pv rotation
        while fill_queue:
            u = fill_queue.popleft()
            if getattr(u, "func", None) is outproj_unit:
                u = partial(outproj_unit, *u.args[:3], pvp)
            u()


def _host_inputs(x, w_qkv, q_bias, v_bias, w_out):
    """Build the 8 per-core input maps."""
    half = D // 2
    # pair-split column permutation within each head's 64 cols
    perm64 = np.empty(D, dtype=np.int64)
    perm64[:half] = 2 * np.arange(half)
    perm64[half:] = 2 * np.arange(half) + 1

    dim_t = (TEMP ** (np.arange(half, dtype=np.float32) / half)).astype(np.float32)
    ang = (np.arange(S, dtype=np.float32)[None, :] / dim_t[:, None]).astype(np.float32)
    cos32 = np.cos(ang).astype(np.float32)      # [32, S]
    sin32 = np.sin(ang).astype(np.float32)
    cosT = np.tile(cos32, (4, 1))               # [128, S]
    sinT = np.concatenate([-sin32, sin32, -sin32, sin32], axis=0)

    import ml_dtypes
    r = np.arange(P)
    tri = np.where(r[None, :] >= r[:, None], 0.0, -1e9).astype(ml_dtypes.bfloat16)
    idn = np.eye(P, dtype=np.float32).astype(ml_dtypes.bfloat16)

    Wq = w_qkv[:, 0:C]
    Wk = w_qkv[:, C:2 * C]
    Wv = w_qkv[:, 2 * C:3 * C]

    in_maps = []
    for core in range(N_CORES):
        b, g = core // HPC, core % HPC
        h0 = HPC * g
        cols = np.concatenate(
            [64 * h + perm64 for h in range(h0, h0 + HPC)]
        )                                        # permuted q/k cols, len 256
        vcols = np.arange(64 * h0, 64 * h0 + 256)
        wv260 = np.zeros((C, 260), dtype=np.float32)
        vb260 = np.zeros((1, 260), dtype=np.float32)
        wvc = Wv[:, vcols]
        vbc = v_bias[vcols]
        for hh_ in range(4):
            wv260[:, 65 * hh_:65 * hh_ + 64] = wvc[:, 64 * hh_:64 * hh_ + 64]
            vb260[0, 65 * hh_:65 * hh_ + 64] = vbc[64 * hh_:64 * hh_ + 64]
            vb260[0, 65 * hh_ + 64] = 1.0
        wv260 = np.ascontiguousarray(wv260)
        def pmaj(a, kt):
            # [kt*128, F] -> partition-major [128, kt, F]
            return np.ascontiguousarray(
                a.reshape(kt, P, -1).transpose(1, 0, 2)).astype(np.float16)

        xh = np.ascontiguousarray(
            x[b].T.reshape(CKT, P, 2, 1024).transpose(1, 2, 0, 3)
        ).astype(np.float16)
        in_maps.append({
            "xT": xh,
            "wq": pmaj(Wq[:, cols], CKT),
            "wk": pmaj(Wk[:, cols], CKT),
            "wv": pmaj(wv260, CKT),
            "wo": pmaj(w_out[64 * h0:64 * h0 + 256, :], 2),
            "qb": np.ascontiguousarray(q_bias[cols].reshape(2, P, 1)),
            "vb": vb260,
            "cosT": cosT.astype(np.float16),
            "sinT": sinT.astype(np.float16),
            "tri": tri, "idn": idn,
        })
    return in_maps


def kernel(x, w_qkv, q_bias, v_bias, w_out, _trace=False):
    global _NC
    if _NC is None:
        _NC = _build()
    in_maps = _host_inputs(
        np.asarray(x, np.float32), np.asarray(w_qkv, np.float32),
        np.asarray(q_bias, np.float32), np.asarray(v_bias, np.float32),
        np.asarray(w_out, np.float32),
    )
    res = run_bass_kernel_spmd(_NC, in_maps, list(range(N_CORES)), trace=_trace)
    out = np.empty((B, S, C), dtype=np.float32)
    for b in range(B):
        acc = res.results[HPC * b]["y"].astype(np.float32)
        for g in range(1, HPC):
            acc = acc + res.results[HPC * b + g]["y"].astype(np.float32)
        out[b] = acc
    if _trace:
        kernel.last_exec_time_ns = res.exec_time_ns
    return out
